# revision 33
# baseline (speedup 1.0000x reference)
"""Distributed causal multi-head attention for TRN2 (8 NeuronCores).

Sharding: tensor-parallel over heads — core c owns heads {2c, 2c+1} for both
batches. QKV projections computed in transposed layout (feature on partitions,
tokens on free axis), attention computed as S.T = K @ Q.T per 128-key block
with softmax denominators obtained by augmenting V with a ones column. Four
segment-split AllToAlls (1024 tokens each) re-shard from head-parallel to
token-parallel as attention progresses; each core then applies softmax
normalization and the output projection for its 4 x 128 tokens.

Queue discipline: sync = evictions + outputs, gpsimd = collectives + post
loads, scalar = exp only (plus startup consts). Emission order interleaves
batch-1 QKV and per-segment post-processing into attention to keep PE dense.
"""

import sys

sys.path.insert(0, "/opt/trn_rl_repo")

import numpy as np
import ml_dtypes

import concourse.bacc as bacc
import concourse.bass as bass
import concourse.mybir as mybir
import concourse.tile as tile
from concourse.bass_utils import run_bass_kernel_spmd

BF16 = mybir.dt.bfloat16
F32 = mybir.dt.float32
NPBF16 = ml_dtypes.bfloat16

B, T, C, H, D = 2, 2048, 1024, 16, 64
NCORES = 8
HPC = H // NCORES          # heads per core = 2
CP = HPC * D               # feature columns per core = 128
TF = B * T                 # flat tokens = 4096
TS = TF // NCORES          # output tokens per core = 512
# segments: (batch, qcs, tokens-per-core); b1 computed in qc order 1,2,3,0 so
# the final A2A is small and its predecessor is covered by remaining compute
SEGS = [
    (0, (0, 1), 128),
    (0, (2, 3), 128),
    (1, (1, 2), 128),
    (1, (0, 3), 128),
]
NSEG = len(SEGS)
OUTOFF = [0, 128, 256, 384]
SEG_OF = {}
for _g, (_b, _qcs, _tps) in enumerate(SEGS):
    for _i, _qc in enumerate(_qcs):
        SEG_OF[(_b, _qc)] = (_g, _i, _tps)
NCB = C // 128             # feature blocks = 8
NQC = T // 512             # q-chunks per batch = 4
NKB = T // 128             # key blocks per batch = 16
SCALE = float(D) ** -0.5
MASKVAL = -30000.0
CHROW = 130                # a2a chunk rows: 128 y + 2 denom


def build_nc():
    nc = bacc.Bacc("TRN2", target_bir_lowering=False, num_devices=NCORES)

    xT = nc.dram_tensor("xT", [C, TF], BF16, kind="ExternalInput")
    # weights pre-packed on host to the on-chip layout [128, NCB, blockcols]
    wqT = nc.dram_tensor("wqT", [128, NCB * CP], BF16, kind="ExternalInput")
    wkT = nc.dram_tensor("wkT", [128, NCB * CP], BF16, kind="ExternalInput")
    wvT = nc.dram_tensor("wvT", [128, NCB * CP], BF16, kind="ExternalInput")
    woT = nc.dram_tensor("woT", [128, NCB * C], BF16, kind="ExternalInput")
    sel = nc.dram_tensor("sel", [H, C], BF16, kind="ExternalInput")
    mtri = nc.dram_tensor("mtri", [128, 128], F32, kind="ExternalInput")
    ident = nc.dram_tensor("ident", [128, 64], BF16, kind="ExternalInput")
    out = nc.dram_tensor("out", [TS, C], F32, kind="ExternalOutput")

    with tile.TileContext(nc) as tc:
        with (
            tc.tile_pool(name="consts", bufs=1) as consts,
            tc.tile_pool(name="xp", bufs=1) as xp,
            tc.tile_pool(name="qkv", bufs=1) as qkv,
            tc.tile_pool(name="work", bufs=1) as work,
            tc.tile_pool(name="ps", bufs=1, space="PSUM") as psp,
            tc.tile_pool(name="dram", bufs=1, space="DRAM") as dram,
        ):
            # ---- weights & constants (single strided DMAs) ----
            wq_sb = consts.tile([128, NCB, CP], BF16)
            wk_sb = consts.tile([128, NCB, CP], BF16)
            wv_sb = consts.tile([128, NCB, CP], BF16)
            start_dmae = [nc.sync, nc.gpsimd]
            x_sb = [xp.tile([128, TF], BF16, name=f"x_sb{cb}") for cb in range(NCB)]
            for cb in range(NCB):  # first token chunk before anything else
                start_dmae[cb % 2].dma_start(
                    x_sb[cb][:, 0:512], xT[128 * cb : 128 * cb + 128, 0:512]
                )
            for i, (w_sb, wdr) in enumerate(
                ((wq_sb, wqT), (wk_sb, wkT), (wv_sb, wvT))
            ):
                start_dmae[i % 2].dma_start(w_sb[:], wdr[:])
            sel_sb = consts.tile([H, C], BF16)
            nc.sync.dma_start(sel_sb[:], sel[:])
            mtri_sb = consts.tile([128, 128], F32)
            nc.gpsimd.dma_start(mtri_sb[:], mtri[:])
            ident_sb = consts.tile([128, 64], BF16)
            nc.gpsimd.dma_start(ident_sb[:], ident[:])
            warm_sb = consts.tile([NCORES, 8], BF16)

            # ---- remaining x loads, chunk-major ----
            for tcn in range(1, 8):
                for cb in range(NCB):
                    start_dmae[(tcn * NCB + cb) % 2].dma_start(
                        x_sb[cb][:, 512 * tcn : 512 * tcn + 512],
                        xT[128 * cb : 128 * cb + 128, 512 * tcn : 512 * tcn + 512],
                    )

            # wo late: not needed until output projection
            wo_sb = consts.tile([128, NCB, C], BF16)
            nc.sync.dma_start(wo_sb[:], woT[:])

            qT_sb = qkv.tile([128, TF], BF16)
            kT_sb = qkv.tile([128, TF], BF16)
            vT_sb = qkv.tile([128, TF], BF16)
            projs = ((wq_sb, qT_sb), (wk_sb, kT_sb), (wv_sb, vT_sb))

            v_sb = [work.tile([128, NKB, 65], BF16, name=f"v_sb{p}") for p in range(4)]

            a2a_in = [
                dram.tile([NCORES * CHROW, SEGS[g][2]], BF16, name=f"a2a_in{g}")
                for g in range(NSEG)
            ]
            a2a_out = [
                dram.tile([NCORES * CHROW, SEGS[g][2]], BF16, name=f"a2a_out{g}")
                for g in range(NSEG)
            ]

            # ---------- emission units ----------
            def qkv_unit(tcn, pi, on_act=False):
                w_sb, oT = projs[pi]
                ps = psp.tile(
                    [128, 512], F32, tag="st", bufs=3,
                    padded_shape=[128, 1024], name="ps_proj",
                )
                for cb in range(NCB):
                    nc.tensor.matmul(
                        ps[:],
                        lhsT=w_sb[:, cb, :],
                        rhs=x_sb[cb][:, 512 * tcn : 512 * tcn + 512],
                        start=(cb == 0),
                        stop=(cb == NCB - 1),
                    )
                if on_act:
                    nc.scalar.copy(oT[:, 512 * tcn : 512 * tcn + 512], ps[:])
                else:
                    nc.vector.tensor_copy(oT[:, 512 * tcn : 512 * tcn + 512], ps[:])

            def vt_unit(pair, kb):
                hh, b = pair % 2, pair // 2
                if kb == 0:
                    nc.vector.memset(v_sb[pair][:, :, 64:65], 1.0)
                t0 = 2048 * b + 128 * kb
                vt_ps = psp.tile([128, 64], BF16, tag="ot", bufs=2, name="vt_ps")
                nc.tensor.transpose(
                    vt_ps[:],
                    vT_sb[64 * hh : 64 * hh + 64, t0 : t0 + 128],
                    ident_sb[64 * hh : 64 * hh + 64, :],
                )
                nc.vector.tensor_copy(v_sb[pair][:, kb, 0:64], vt_ps[:])

            # attention state per (pair, qc), lives across kbp units
            attn_ot = {}
            pending_pv = []

            def emit_pv(job):
                pair, qc, kbp, pT, offs = job
                ot = attn_ot[(pair, qc)]
                n_kb = 4 * qc + 4
                for h2 in range(2):
                    kb = 2 * kbp + h2
                    off = offs[h2]
                    nc.tensor.matmul(
                        ot[:, off:512],
                        lhsT=v_sb[pair][:, kb, :],
                        rhs=pT[:, 512 * h2 + off : 512 * h2 + 512],
                        start=(kb == 0),
                        stop=(kb == n_kb - 1),
                    )

            def attn_unit(pair, qc, kbp):
                hh, b = pair % 2, pair // 2
                hs = slice(64 * hh, 64 * hh + 64)
                tb0 = 2048 * b
                q0 = tb0 + 512 * qc
                if kbp == 0:
                    attn_ot[(pair, qc)] = psp.tile(
                        [65, 512], F32, tag="ot", bufs=2, name="ot_ps"
                    )
                st = psp.tile([128, 1024], F32, tag="st", bufs=3, name="st_ps")
                offs = []
                for h2 in range(2):
                    kb = 2 * kbp + h2
                    off = max(0, 128 * kb - 512 * qc)
                    offs.append(off)
                    nc.tensor.matmul(
                        st[:, 512 * h2 + off : 512 * h2 + 512],
                        lhsT=kT_sb[hs, tb0 + 128 * kb : tb0 + 128 * kb + 128],
                        rhs=qT_sb[hs, q0 + off : q0 + 512],
                        start=True,
                        stop=True,
                    )
                for h2 in range(2):
                    kb = 2 * kbp + h2
                    if 128 * kb >= 512 * qc:  # diagonal block: triangular mask
                        off = offs[h2]
                        dd = slice(512 * h2 + off, 512 * h2 + off + 128)
                        nc.vector.tensor_add(st[:, dd], st[:, dd], mtri_sb[:])
                pT = work.tile([128, 1024], BF16, tag="pT", bufs=6, name="pT")
                o0 = offs[0]
                nc.scalar.activation(
                    pT[:, o0:1024],
                    st[:, o0:1024],
                    mybir.ActivationFunctionType.Exp,
                    scale=SCALE,
                )
                pending_pv.append((pair, qc, kbp, pT, offs))
                if len(pending_pv) > 3:
                    emit_pv(pending_pv.pop(0))

            def evict_unit(pair, qc):
                while pending_pv:
                    emit_pv(pending_pv.pop(0))
                hh, b = pair % 2, pair // 2
                g, qi, tps = SEG_OF[(b, qc)]
                nch = 512 // tps          # chunks this eviction covers
                s0 = nch * qi
                ot = attn_ot.pop((pair, qc))
                y_sb = work.tile([65, 512], BF16, tag="y", bufs=6, name="y_sb")
                nc.vector.tensor_copy(y_sb[:], ot[:])
                # one DMA: 64 y rows + denom row into block [65*hh .. 65*hh+65)
                ydst = bass.AP(
                    a2a_in[g].tensor,
                    (s0 * CHROW + 65 * hh) * tps,
                    [[tps, 65], [CHROW * tps, nch], [1, tps]],
                )
                nc.sync.dma_start(ydst, y_sb[:, :])

            def coll_unit(g):
                nc.gpsimd.collective_compute(
                    "AllToAll",
                    mybir.AluOpType.bypass,
                    replica_groups=[list(range(NCORES))],
                    ins=[a2a_in[g][:].opt()],
                    outs=[a2a_out[g][:].opt()],
                )

            def post_dma(g):
                tps = SEGS[g][2]
                den_sb = work.tile([H, tps], BF16, tag="den", bufs=2, name="den_sb")
                dsrc = bass.AP(
                    a2a_out[g].tensor,
                    64 * tps,
                    [[CHROW * tps, NCORES], [65 * tps, 2], [1, tps]],
                )
                nc.sync.dma_start(den_sb[:], dsrc)
                y_loc = [
                    work.tile([128, tps], BF16, tag=f"yloc{cb}", bufs=2, name=f"y_loc{cb}")
                    for cb in range(NCB)
                ]
                for cb in range(NCB):
                    nc.sync.dma_start(
                        y_loc[cb][0:64, :], a2a_out[g][CHROW * cb : CHROW * cb + 64, :]
                    )
                    nc.sync.dma_start(
                        y_loc[cb][64:128, :],
                        a2a_out[g][CHROW * cb + 65 : CHROW * cb + 129, :],
                    )
                return y_loc, den_sb

            def post_compute(g, y_loc, den_sb):
                tps = SEGS[g][2]
                recip = work.tile([H, tps], F32, tag="recip", bufs=2, name="recip")
                nc.vector.reciprocal(recip[:], den_sb[:])
                recip_bf = work.tile([H, tps], BF16, tag="recipbf", bufs=2, name="recip_bf")
                nc.vector.tensor_copy(recip_bf[:], recip[:])
                yn = [
                    work.tile([128, tps], BF16, tag=f"yn{cb}", bufs=2, name=f"yn{cb}")
                    for cb in range(NCB)
                ]

                def norm_unit(cb):
                    bc_ps = psp.tile([128, tps], F32, tag="st", bufs=3, padded_shape=[128, 1024], name="bc_ps")
                    nc.tensor.matmul(
                        bc_ps[:],
                        lhsT=sel_sb[:, 128 * cb : 128 * cb + 128],
                        rhs=recip_bf[:],
                        start=True,
                        stop=True,
                    )
                    nc.vector.tensor_mul(yn[cb][:], y_loc[cb][:], bc_ps[:])

                def proj_unit(tb, mh):
                    ps = psp.tile(
                        [128, 512], F32, tag="st", bufs=3,
                        padded_shape=[128, 1024], name="ps_op",
                    )
                    for cb in range(NCB):
                        nc.tensor.matmul(
                            ps[:],
                            lhsT=yn[cb][:, 128 * tb : 128 * tb + 128],
                            rhs=wo_sb[:, cb, 512 * mh : 512 * mh + 512],
                            start=(cb == 0),
                            stop=(cb == NCB - 1),
                        )
                    o_sb = work.tile([128, 512], F32, tag="osb", bufs=3, name="o_sb")
                    nc.vector.tensor_copy(o_sb[:], ps[:])
                    nc.sync.dma_start(
                        out[
                            OUTOFF[g] + 128 * tb : OUTOFF[g] + 128 * tb + 128,
                            512 * mh : 512 * mh + 512,
                        ],
                        o_sb[:],
                    )

                units = [("f", norm_unit, (cb,)) for cb in range(NCB)]
                units += [
                    ("f", proj_unit, (tb, mh))
                    for tb in range(tps // 128)
                    for mh in range(2)
                ]
                return units

            def warmup_coll():
                win = dram.tile([NCORES, 8], BF16, name="warm_in")
                wout = dram.tile([NCORES, 8], BF16, name="warm_out")
                nc.vector.memset(warm_sb[:], 0.0)
                nc.sync.dma_start(win[:], warm_sb[:])
                nc.gpsimd.collective_compute(
                    "AllToAll",
                    mybir.AluOpType.bypass,
                    replica_groups=[list(range(NCORES))],
                    ins=[win[:].opt()],
                    outs=[wout[:].opt()],
                )

            def attn_units_for_seg(g):
                b, qcs, _ = SEGS[g]
                units = []
                for qc in qcs:
                    for hh in range(2):
                        pair = 2 * b + hh
                        for kbp in range(2 * qc + 2):
                            units.append(("a", pair, qc, kbp))
                        units.append(("e", pair, qc))
                return units

            def run_unit(u):
                if u[0] == "a":
                    attn_unit(u[1], u[2], u[3])
                elif u[0] == "e":
                    evict_unit(u[1], u[2])
                elif u[0] == "q":
                    qkv_unit(u[1], u[2])
                elif u[0] == "v":
                    vt_unit(u[1], u[2])
                elif u[0] == "f":
                    u[1](*u[2])

            feed = []
            fi = 0

            def run_chunk(main, inject_at=None, inject_units=None):
                # feed is consumed in bursts of 2 so the PE gets ~3.4us of
                # contiguous independent matmuls (a full HAM activity window)
                nonlocal fi
                for i, u in enumerate(main):
                    if inject_at is not None and i == inject_at:
                        feed.extend(inject_units)
                    run_unit(u)
                    if fi < len(feed):
                        run_unit(feed[fi])
                        fi += 1

            def run_feed_one():
                nonlocal fi
                if fi < len(feed):
                    run_unit(feed[fi])
                    fi += 1
                    return True
                return False

            def drain_feed():
                nonlocal fi
                while fi < len(feed):
                    run_unit(feed[fi])
                    fi += 1

            # ---------- schedule ----------
            # minimal prologue: only what attention (b0, qc0) needs up-front
            for pi in range(3):
                qkv_unit(0, pi)
            for hh in range(2):
                for kb in range(4):
                    vt_unit(hh, kb)

            feed += [("q", 1, pi) for pi in range(3)]
            feed += [("v", hh, kb) for hh in range(2) for kb in range(4, 8)]
            feed += [("q", tcn, pi) for tcn in (2, 3) for pi in range(3)]
            feed += [("v", hh, kb) for hh in range(2) for kb in range(8, NKB)]
            feed += [("q", tcn, pi) for tcn in range(4, 8) for pi in range(3)]
            feed += [("v", 2 + hh, kb) for hh in range(2) for kb in range(NKB)]

            run_chunk(attn_units_for_seg(0))              # b0 qc0-1
            coll_unit(0)
            run_chunk(attn_units_for_seg(1))              # b0 qc2-3
            coll_unit(1)
            drain_feed()  # all b1 qkv/vt must be emitted before b1 attention
            main2 = attn_units_for_seg(2)                 # b1 qc1-2
            for i, u in enumerate(main2):
                if i == 20:
                    # post-collective DMAs deferred here so segment-1
                    # evictions on sync were never queued behind them
                    feed.extend(post_compute(0, *post_dma(0)))
                run_unit(u)
                run_feed_one()
            drain_feed()
            coll_unit(2)
            main3 = attn_units_for_seg(3)                 # b1 qc0 then qc3
            pc1 = pc2 = None
            for i, u in enumerate(main3):
                if i == 6:
                    pc1 = post_compute(1, *post_dma(1))
                    feed.extend(pc1[:NCB])  # norm units only
                if i == 16:
                    pc2 = post_compute(2, *post_dma(2))
                    feed.extend(pc2[:NCB])
                run_unit(u)
                run_feed_one()
            drain_feed()
            coll_unit(3)
            pd3 = post_dma(3)
            # held-back projections fill the final collective's flight time
            for u in pc1[NCB:]:
                run_unit(u)
            for u in pc2[NCB:]:
                run_unit(u)
            for u in post_compute(3, *pd3):
                run_unit(u)

    nc.compile()
    return nc


_NC = None


def _get_nc():
    global _NC
    if _NC is None:
        _NC = build_nc()
    return _NC


def _host_consts():
    sel = np.zeros((H, C), dtype=np.float32)
    for h in range(H):
        sel[h, 64 * h : 64 * h + 64] = 1.0
    idx = np.arange(128)
    mtri = np.where(idx[None, :] >= idx[:, None], 0.0, MASKVAL).astype(np.float32)
    ident = np.concatenate([np.eye(64, dtype=np.float32)] * 2, axis=0)
    return sel.astype(NPBF16), mtri, ident.astype(NPBF16)


def _pack_w(wT):
    # [C, cols] -> [128, NCB*cols]: w_sb[p, cb*cols+j] = wT[128*cb+p, j]
    cols = wT.shape[1]
    return np.ascontiguousarray(
        wT.reshape(NCB, 128, cols).transpose(1, 0, 2).reshape(128, NCB * cols)
    )


def _make_in_maps(x, Wq, Wk, Wv, Wo):
    xT = np.ascontiguousarray(x.reshape(TF, C).T).astype(NPBF16)
    woT = _pack_w(Wo.T).astype(NPBF16)
    sel, mtri, ident = _host_consts()
    in_maps = []
    for c in range(NCORES):
        rows = slice(CP * c, CP * c + CP)
        in_maps.append(
            {
                "xT": xT,
                "wqT": _pack_w(Wq[rows].T).astype(NPBF16),
                "wkT": _pack_w(Wk[rows].T).astype(NPBF16),
                "wvT": _pack_w(Wv[rows].T).astype(NPBF16),
                "woT": woT,
                "sel": sel,
                "mtri": mtri,
                "ident": ident,
            }
        )
    return in_maps


def _assemble(results):
    full = np.zeros((TF, C), dtype=np.float32)
    for c in range(NCORES):
        o = results[c]["out"]
        for g in range(NSEG):
            b, qcs, tps = SEGS[g]
            cpq = 512 // tps  # chunks per q-chunk
            qc = qcs[c // cpq]
            base = 2048 * b + 512 * qc + tps * (c % cpq)
            full[base : base + tps] = o[OUTOFF[g] : OUTOFF[g] + tps]
    return full.reshape(B, T, C)


def kernel(x, mask, Wq, Wk, Wv, Wo):
    del mask  # causal mask is hardcoded in the device kernel
    in_maps = _make_in_maps(
        np.asarray(x, dtype=np.float32),
        np.asarray(Wq, dtype=np.float32),
        np.asarray(Wk, dtype=np.float32),
        np.asarray(Wv, dtype=np.float32),
        np.asarray(Wo, dtype=np.float32),
    )
    nc = _get_nc()
    res = run_bass_kernel_spmd(nc, in_maps, core_ids=list(range(NCORES)))
    return _assemble(res.results)
